# revision 26
# baseline (speedup 1.0000x reference)
"""BERT forward on 8 Trainium2 NeuronCores.

Sharding: data-parallel over batch (B=16 -> 2 sequences per core).
Each core runs the full 12-layer encoder + NSP + MLM heads for its 2
sequences, including the tied-vocab decoder (streams tok_embed.T from HBM).

Layout strategy per layer (per batch, S=512 tokens):
  - residual x kept fp32, "normal" layout [token, 768] (4 tiles of [128, 768])
  - x cast to bf16 and PE-transposed to xT [768, 512] for the projections
  - Q/K projections produce qT/kT [head*64, 512] (weights stationary)
  - V projection produces v in normal layout [token, 768] (xT stationary),
    stored per k-tile/head with a ones column -> the attn@V matmul also
    produces the softmax denominator row (no separate sum pass)
  - scoresT [kpos, q] = kT_h^T-free matmul; exp on ScalarE with the padding
    mask folded into the per-partition bias and 1/8 scale folded into scale
  - ctx~T [65, 512] accumulated in PSUM; row 64 = sum; reciprocal +
    gpsimd partition_broadcast + DVE multiply -> normalized ctxT (bf16)
  - out-proj: ctxT stationary, Wo moving -> y in normal layout; bias bo via
    a K=1 ones-row matmul; residual add + LayerNorm (bn_stats) in fp32
Heads:
  - masked-position + CLS gather via one-hot selection matmuls (no DRAM RT dep)
  - tied decoder: h2T stationary, host-pretransposed tok_embed.T streamed as
    the moving operand; dec_bias via K=1 ones matmul
"""

import os

import numpy as np
import ml_dtypes

B, S, D, H, DK, L, V, PM, NSEG = 16, 512, 768, 12, 64, 12, 32000, 20, 2
DC = D // 128          # 6 contraction chunks
TT = S // 128          # 4 token tiles
BPC = 2                # batches per core
NCORES = 8
JCOL = PM + 1          # 21 selection cols per batch (20 masked + CLS)
NL = int(os.environ.get("BERT_NLAYERS", str(L)))  # debug override

_BUILD_CACHE = {}


def _build(apply_gb: bool):
    import concourse.bass as bass
    import concourse.tile as tile
    import concourse.mybir as mybir
    from concourse import bacc
    from concourse.masks import make_identity

    f32 = mybir.dt.float32
    bf16 = mybir.dt.bfloat16
    i32 = mybir.dt.int32
    ALU = mybir.AluOpType
    ACTF = mybir.ActivationFunctionType

    nc = bacc.Bacc(
        "TRN2", target_bir_lowering=False, debug=False, num_devices=NCORES
    )

    # ---- per-core inputs ----
    embtok_in = nc.dram_tensor("embtok", [BPC, S, D], f32, kind="ExternalInput")
    segsel_in = nc.dram_tensor("segsel", [NSEG, BPC, TT, 128], bf16, kind="ExternalInput")
    maskb_in = nc.dram_tensor("maskb", [128, BPC, TT], f32, kind="ExternalInput")
    sel_in = nc.dram_tensor("sel", [BPC, TT, 128, JCOL], bf16, kind="ExternalInput")
    pos_in = nc.dram_tensor("pos_embed", [S, D], bf16, kind="ExternalInput")
    sege_in = nc.dram_tensor("seg_row", [NSEG, D], bf16, kind="ExternalInput")
    wq_in = nc.dram_tensor("wq", [L, DC, 128, D], bf16, kind="ExternalInput")
    wk_in = nc.dram_tensor("wk", [L, DC, 128, D], bf16, kind="ExternalInput")
    wv_in = nc.dram_tensor("wv", [L, DC, 128, D], bf16, kind="ExternalInput")
    wo_in = nc.dram_tensor("wo", [L, DC, 128, D], bf16, kind="ExternalInput")
    bqk_in = nc.dram_tensor("bqk", [128, L, 2, DC], f32, kind="ExternalInput")
    bvo_in = nc.dram_tensor("bvo", [L, 2, D], bf16, kind="ExternalInput")
    lng_in = nc.dram_tensor("lng", [L, D], f32, kind="ExternalInput")
    lnb_in = nc.dram_tensor("lnb", [L, D], f32, kind="ExternalInput")
    embg_in = nc.dram_tensor("embg", [1, D], f32, kind="ExternalInput")
    embb_in = nc.dram_tensor("embb", [1, D], f32, kind="ExternalInput")
    fcw_in = nc.dram_tensor("fcw", [DC, 128, D], bf16, kind="ExternalInput")
    fcb_in = nc.dram_tensor("fcb", [128, DC], f32, kind="ExternalInput")
    clsw_in = nc.dram_tensor("clsw", [DC, 128, 2], bf16, kind="ExternalInput")
    clsb_in = nc.dram_tensor("clsb", [2, 1], f32, kind="ExternalInput")
    mlmw_in = nc.dram_tensor("mlmw", [DC, 128, D], bf16, kind="ExternalInput")
    mlmb_in = nc.dram_tensor("mlmb", [128, DC], f32, kind="ExternalInput")
    mlng_in = nc.dram_tensor("mlng", [1, D], f32, kind="ExternalInput")
    mlnb_in = nc.dram_tensor("mlnb", [1, D], f32, kind="ExternalInput")
    decb_in = nc.dram_tensor("decb", [1, V], bf16, kind="ExternalInput")
    embT_in = nc.dram_tensor("embT", [DC, 128, V], bf16, kind="ExternalInput")

    # ---- per-core outputs ----
    x_out = nc.dram_tensor("x_out", [BPC, S, D], f32, kind="ExternalOutput")
    lg_out = nc.dram_tensor("lg_out", [BPC * PM, V], f32, kind="ExternalOutput")
    nsp_out = nc.dram_tensor("nsp_out", [2, BPC], f32, kind="ExternalOutput")
    DBG = bool(int(os.environ.get("BERT_DEBUG", "0")))
    if DBG:
        dbg_haug = nc.dram_tensor("dbg_haug", [128, DC, BPC * JCOL], bf16, kind="ExternalOutput")
        dbg_h1gT = nc.dram_tensor("dbg_h1gT", [128, DC, BPC * PM], f32, kind="ExternalOutput")
        dbg_h2 = nc.dram_tensor("dbg_h2", [BPC * PM, D], f32, kind="ExternalOutput")
        dbg_hp = nc.dram_tensor("dbg_hp", [128, DC, BPC], bf16, kind="ExternalOutput")

    with tile.TileContext(nc) as tc:
        import contextlib

        ctx = contextlib.ExitStack()
        with ctx:
            # SBUF pools
            persist = ctx.enter_context(tc.tile_pool(name="persist", bufs=1))
            wpool = ctx.enter_context(tc.tile_pool(name="wpool", bufs=1))
            small = ctx.enter_context(tc.tile_pool(name="small", bufs=3))
            embp = ctx.enter_context(tc.tile_pool(name="embp", bufs=2))
            xbfp = ctx.enter_context(tc.tile_pool(name="xbfp", bufs=3))
            expp = ctx.enter_context(tc.tile_pool(name="expp", bufs=2))
            bcp = ctx.enter_context(tc.tile_pool(name="bcp", bufs=2))
            etp = ctx.enter_context(tc.tile_pool(name="etp", bufs=2))
            lgp = ctx.enter_context(tc.tile_pool(name="lgp", bufs=2))
            # PSUM pools: "ps" [128,512]x4 banks + "wide" [128,768]x2 = 8 banks
            ps = ctx.enter_context(tc.tile_pool(name="ps", bufs=4, space="PSUM"))
            wps = ctx.enter_context(tc.tile_pool(name="wps", bufs=2, space="PSUM"))

            # ---- persistent SBUF state ----
            x = persist.tile([128, BPC, TT, D], f32, tag="x")
            xT = persist.tile([128, BPC, DC, S], bf16, tag="xT")
            qT = persist.tile([128, BPC, DC, S], bf16, tag="qT")
            kT = persist.tile([128, BPC, DC, S], bf16, tag="kT")
            vbuf = persist.tile([128, BPC, TT, H, 66], bf16, tag="vbuf")
            ctxT = persist.tile([128, BPC, DC, S], bf16, tag="ctxT")
            ident_f = persist.tile([128, 128], f32, tag="idf")
            ident_b = persist.tile([128, 128], bf16, tag="idb")
            ones_b = persist.tile([1, 128], bf16, tag="ones")
            eps_t = persist.tile([128, 1], f32, tag="eps")
            bqk_sb = persist.tile([128, L, 2, DC], f32, tag="bqk")
            maskb_sb = persist.tile([128, BPC, TT], f32, tag="maskb")
            segsel_sb = persist.tile([NSEG, BPC, TT, 128], bf16, tag="segsel")
            sege_sb = persist.tile([NSEG, D], bf16, tag="serow")
            posbf_sb = persist.tile([128, TT, D], bf16, tag="posbf")
            sel_sb = persist.tile([128, BPC, TT, JCOL], bf16, tag="sel")
            haugT = persist.tile([128, DC, BPC * JCOL], bf16, tag="haugT")
            hp_sb = persist.tile([128, DC, BPC], bf16, tag="hp")
            h1gT = persist.tile([128, DC, BPC * PM], f32, tag="h1gT")
            h1g = persist.tile([BPC * PM, D], f32, tag="h1g")
            h2bf = persist.tile([BPC * PM, D], bf16, tag="h2bf")
            h2T = persist.tile([128, DC, BPC * PM], bf16, tag="h2T")
            fcb_sb = persist.tile([128, DC], f32, tag="fcb")
            clsb_sb = persist.tile([2, 1], f32, tag="clsb")
            mlmb_sb = persist.tile([128, DC], f32, tag="mlmb")
            gb_sb = None
            if apply_gb:
                gb_sb = persist.tile([128, 2, D], f32, tag="gb")  # g,b bcast

            make_identity(nc, ident_f)
            nc.vector.tensor_copy(out=ident_b, in_=ident_f)
            nc.vector.memset(ones_b, 1.0)
            nc.vector.memset(eps_t, 1e-5)
            nc.vector.memset(vbuf[:, :, :, :, 64:65], 1.0)
            nc.sync.dma_start(out=bqk_sb, in_=bqk_in[:])
            nc.sync.dma_start(out=maskb_sb, in_=maskb_in[:])
            nc.sync.dma_start(out=segsel_sb, in_=segsel_in[:])
            nc.sync.dma_start(out=sege_sb, in_=sege_in[:])
            nc.sync.dma_start(
                out=posbf_sb,
                in_=pos_in[:].rearrange("(t p) d -> p t d", p=128),
            )
            nc.sync.dma_start(
                out=sel_sb, in_=sel_in[:].rearrange("b t p j -> p b t j")
            )
            nc.sync.dma_start(out=fcb_sb, in_=fcb_in[:])
            nc.sync.dma_start(out=clsb_sb, in_=clsb_in[:])
            nc.sync.dma_start(out=mlmb_sb, in_=mlmb_in[:])

            def layernorm(xap, g_row=None, b_row=None, gbtile=None):
                """In-place LN over last dim (free) of fp32 [p, 768] AP."""
                p = xap.shape[0]
                resh = xap.rearrange("p (s f) -> p s f", f=256)
                stats = small.tile([128, 3, 6], f32, tag="stats")
                mv = small.tile([128, 2], f32, tag="mv")
                for s3 in range(3):
                    nc.vector.bn_stats(out=stats[:p, s3, :], in_=resh[:, s3, :])
                nc.vector.bn_aggr(out=mv[:p], in_=stats[:p])
                std = small.tile([128, 1], f32, tag="std")
                nc.scalar.activation(
                    out=std[:p], in_=mv[:p, 1:2], func=ACTF.Sqrt, bias=eps_t[:p]
                )
                nc.vector.reciprocal(out=std[:p], in_=std[:p])
                nc.vector.tensor_scalar(
                    out=xap,
                    in0=xap,
                    scalar1=mv[:p, 0:1],
                    scalar2=std[:p],
                    op0=ALU.subtract,
                    op1=ALU.mult,
                )
                if apply_gb:
                    assert gbtile is not None
                    if g_row is not None:
                        nc.gpsimd.partition_broadcast(gbtile[:, 0, :], g_row)
                        nc.gpsimd.partition_broadcast(gbtile[:, 1, :], b_row)
                    nc.vector.tensor_tensor(
                        out=xap, in0=xap, in1=gbtile[:p, 0, :], op=ALU.mult
                    )
                    nc.vector.tensor_tensor(
                        out=xap, in0=xap, in1=gbtile[:p, 1, :], op=ALU.add
                    )

            # ================= embeddings =================
            if apply_gb:
                grow = small.tile([1, D], f32, tag="grow")
                brow = small.tile([1, D], f32, tag="brow")
                nc.sync.dma_start(out=grow, in_=embg_in[:])
                nc.sync.dma_start(out=brow, in_=embb_in[:])
            for b in range(BPC):
                for t in range(TT):
                    emb = embp.tile([128, D], f32, tag="emb")
                    nc.sync.dma_start(out=emb, in_=embtok_in[b, t * 128 : (t + 1) * 128, :])
                    sp = wps.tile([128, D], f32, tag="wide", space="PSUM")
                    for n0, n1 in ((0, 512), (512, 768)):
                        nc.tensor.matmul(
                            out=sp[:, n0:n1],
                            lhsT=segsel_sb[:, b, t, :],
                            rhs=sege_sb[:, n0:n1],
                            start=True,
                            stop=False,
                        )
                        nc.tensor.matmul(
                            out=sp[:, n0:n1],
                            lhsT=ident_b,
                            rhs=posbf_sb[:, t, n0:n1],
                            start=False,
                            stop=True,
                        )
                    nc.vector.tensor_copy(out=x[:, b, t, :], in_=emb)
                    nc.vector.tensor_tensor(
                        out=x[:, b, t, :], in0=x[:, b, t, :], in1=sp, op=ALU.add
                    )
                    if apply_gb:
                        layernorm(
                            x[:, b, t, :],
                            grow if (b == 0 and t == 0) else None,
                            brow if (b == 0 and t == 0) else None,
                            gbtile=gb_sb,
                        )
                    else:
                        layernorm(x[:, b, t, :])

            # ================= encoder layers =================
            for l in range(NL):
                wq = wpool.tile([128, DC, D], bf16, tag="wq")
                wk = wpool.tile([128, DC, D], bf16, tag="wk")
                wv = wpool.tile([128, DC, D], bf16, tag="wv")
                wo = wpool.tile([128, DC, D], bf16, tag="wo")
                bvo = wpool.tile([1, 2, D], bf16, tag="bvo")
                nc.sync.dma_start(out=wq, in_=wq_in[l].rearrange("c p d -> p c d"))
                nc.sync.dma_start(out=wk, in_=wk_in[l].rearrange("c p d -> p c d"))
                nc.sync.dma_start(out=wv, in_=wv_in[l].rearrange("c p d -> p c d"))
                nc.sync.dma_start(out=wo, in_=wo_in[l].rearrange("c p d -> p c d"))
                nc.sync.dma_start(out=bvo, in_=bvo_in[l][None])
                if apply_gb:
                    grow_l = small.tile([1, D], f32, tag="grow")
                    brow_l = small.tile([1, D], f32, tag="brow")
                    nc.sync.dma_start(out=grow_l, in_=lng_in[l : l + 1, :])
                    nc.sync.dma_start(out=brow_l, in_=lnb_in[l : l + 1, :])

                for b in range(BPC):
                    # ---- transpose x -> xT (bf16) ----
                    for t in range(TT):
                        xbf = xbfp.tile([128, D], bf16, tag="xbf")
                        nc.vector.tensor_copy(out=xbf, in_=x[:, b, t, :])
                        for c in range(DC):
                            tp = ps.tile([128, 128], bf16, tag="ps", space="PSUM")
                            nc.tensor.transpose(
                                tp, xbf[:, c * 128 : (c + 1) * 128], ident_b
                            )
                            nc.vector.tensor_copy(
                                out=xT[:, b, c, t * 128 : (t + 1) * 128], in_=tp
                            )
                    # ---- Q/K projections -> qT/kT ----
                    for wi, (wt, dst) in enumerate(((wq, qT), (wk, kT))):
                        for mt in range(DC):
                            pt = ps.tile([128, S], f32, tag="ps", space="PSUM")
                            for c in range(DC):
                                nc.tensor.matmul(
                                    out=pt,
                                    lhsT=wt[:, c, mt * 128 : (mt + 1) * 128],
                                    rhs=xT[:, b, c, :],
                                    start=(c == 0),
                                    stop=(c == DC - 1),
                                )
                            nc.vector.tensor_scalar_add(
                                out=dst[:, b, mt, :],
                                in0=pt,
                                scalar1=bqk_sb[:, l, wi, mt : mt + 1],
                            )
                    # ---- V projection (normal layout) + bias ----
                    for t in range(TT):
                        yp = wps.tile([128, D], f32, tag="wide", space="PSUM")
                        for half, n0, n1 in ((0, 0, 512), (1, 512, 768)):
                            nc.tensor.matmul(
                                out=yp[:, n0:n1],
                                lhsT=ones_b,
                                rhs=bvo[:, 0, n0:n1],
                                start=True,
                                stop=False,
                            )
                            for c in range(DC):
                                nc.tensor.matmul(
                                    out=yp[:, n0:n1],
                                    lhsT=xT[:, b, c, t * 128 : (t + 1) * 128],
                                    rhs=wv[:, c, n0:n1],
                                    start=False,
                                    stop=(c == DC - 1),
                                )
                        nc.vector.tensor_copy(
                            out=vbuf[:, b, t, :, 0:64],
                            in_=yp.rearrange("p (h e) -> p h e", e=64),
                        )
                    # ---- attention per head ----
                    for h in range(H):
                        mt, half = h // 2, h % 2
                        qh = qT[64 * half : 64 * half + 64, b, mt, :]
                        kh = kT[64 * half : 64 * half + 64, b, mt, :]
                        ex = expp.tile([128, TT, S], bf16, tag="expT")
                        for t in range(TT):
                            sc = ps.tile([128, S], f32, tag="ps", space="PSUM")
                            nc.tensor.matmul(
                                out=sc,
                                lhsT=kh[:, t * 128 : (t + 1) * 128],
                                rhs=qh,
                                start=True,
                                stop=True,
                            )
                            nc.scalar.activation(
                                out=ex[:, t, :],
                                in_=sc,
                                func=ACTF.Exp,
                                bias=maskb_sb[:, b, t : t + 1],
                                scale=0.125,
                            )
                        cp = ps.tile([65, S], f32, tag="ps", space="PSUM")
                        for t in range(TT):
                            nc.tensor.matmul(
                                out=cp,
                                lhsT=vbuf[:, b, t, h, 0:65],
                                rhs=ex[:, t, :],
                                start=(t == 0),
                                stop=(t == TT - 1),
                            )
                        rc = small.tile([1, S], f32, tag="rc")
                        nc.vector.reciprocal(out=rc, in_=cp[64:65, :])
                        bc = bcp.tile([64, S], f32, tag="bc")
                        nc.gpsimd.partition_broadcast(bc, rc)
                        nc.vector.tensor_tensor(
                            out=ctxT[64 * half : 64 * half + 64, b, mt, :],
                            in0=cp[0:64, :],
                            in1=bc,
                            op=ALU.mult,
                        )
                    # ---- out projection + residual + LN ----
                    for t in range(TT):
                        yp = wps.tile([128, D], f32, tag="wide", space="PSUM")
                        for half, n0, n1 in ((0, 0, 512), (1, 512, 768)):
                            nc.tensor.matmul(
                                out=yp[:, n0:n1],
                                lhsT=ones_b,
                                rhs=bvo[:, 1, n0:n1],
                                start=True,
                                stop=False,
                            )
                            for c in range(DC):
                                nc.tensor.matmul(
                                    out=yp[:, n0:n1],
                                    lhsT=ctxT[:, b, c, t * 128 : (t + 1) * 128],
                                    rhs=wo[:, c, n0:n1],
                                    start=False,
                                    stop=(c == DC - 1),
                                )
                        nc.vector.tensor_tensor(
                            out=x[:, b, t, :], in0=yp, in1=x[:, b, t, :], op=ALU.add
                        )
                        if apply_gb:
                            layernorm(
                                x[:, b, t, :],
                                grow_l if (b == 0 and t == 0) else None,
                                brow_l if (b == 0 and t == 0) else None,
                                gbtile=gb_sb,
                            )
                        else:
                            layernorm(x[:, b, t, :])

            # ================= outputs: x, heads =================
            for b in range(BPC):
                for t in range(TT):
                    nc.sync.dma_start(
                        out=x_out[b, t * 128 : (t + 1) * 128, :], in_=x[:, b, t, :]
                    )
            # selection matmuls: haugT[d, b*21+j] = sum_t x[t, d] * sel[t, j]
            xself = persist.tile([128, TT, D], bf16, tag="xself")
            for b in range(BPC):
                for t in range(TT):
                    nc.vector.tensor_copy(out=xself[:, t, :], in_=x[:, b, t, :])
                for c in range(DC):
                    hs = ps.tile([128, JCOL], f32, tag="ps", space="PSUM")
                    for t in range(TT):
                        nc.tensor.matmul(
                            out=hs,
                            lhsT=xself[:, t, c * 128 : (c + 1) * 128],
                            rhs=sel_sb[:, b, t, :],
                            start=(t == 0),
                            stop=(t == TT - 1),
                        )
                    nc.vector.tensor_copy(
                        out=haugT[:, c, b * JCOL : (b + 1) * JCOL], in_=hs
                    )
            if DBG:
                nc.sync.dma_start(out=dbg_haug[:], in_=haugT)
            # ---- NSP head ----
            fcw = wpool.tile([128, DC, D], bf16, tag="wq")  # reuse weight slot
            nc.sync.dma_start(out=fcw, in_=fcw_in[:].rearrange("c p d -> p c d"))
            cls_cols = haugT[:, :, PM :: JCOL]  # [128, DC, BPC] CLS columns
            for mt in range(DC):
                np_ = ps.tile([128, BPC], f32, tag="ps", space="PSUM")
                for c in range(DC):
                    nc.tensor.matmul(
                        out=np_,
                        lhsT=fcw[:, c, mt * 128 : (mt + 1) * 128],
                        rhs=cls_cols[:, c, :],
                        start=(c == 0),
                        stop=(c == DC - 1),
                    )
                nc.scalar.activation(
                    out=hp_sb[:, mt, :],
                    in_=np_,
                    func=ACTF.Tanh,
                    bias=fcb_sb[:, mt : mt + 1],
                )
            clsw = wpool.tile([128, DC, 2], bf16, tag="clsw")
            nc.sync.dma_start(out=clsw, in_=clsw_in[:].rearrange("c p d -> p c d"))
            nsp_ps = ps.tile([2, BPC], f32, tag="ps", space="PSUM")
            for c in range(DC):
                nc.tensor.matmul(
                    out=nsp_ps,
                    lhsT=clsw[:, c, :],
                    rhs=hp_sb[:, c, :],
                    start=(c == 0),
                    stop=(c == DC - 1),
                )
            nsp_sb = small.tile([2, BPC], f32, tag="nsp")
            nc.vector.tensor_scalar_add(
                out=nsp_sb, in0=nsp_ps, scalar1=clsb_sb
            )
            nc.sync.dma_start(out=nsp_out[:], in_=nsp_sb)
            # ---- MLM transform ----
            mlmw = wpool.tile([128, DC, D], bf16, tag="wk")  # reuse slot
            nc.sync.dma_start(out=mlmw, in_=mlmw_in[:].rearrange("c p d -> p c d"))
            hmT = haugT.rearrange("p c (b j) -> p c b j", j=JCOL)[:, :, :, 0:PM]
            for mt in range(DC):
                mp = ps.tile([128, BPC * PM], f32, tag="ps", space="PSUM")
                for c in range(DC):
                    nc.tensor.matmul(
                        out=mp,
                        lhsT=mlmw[:, c, mt * 128 : (mt + 1) * 128],
                        rhs=hmT[:, c],
                        start=(c == 0),
                        stop=(c == DC - 1),
                    )
                nc.scalar.activation(
                    out=h1gT[:, mt, :],
                    in_=mp,
                    func=ACTF.Gelu,
                    bias=mlmb_sb[:, mt : mt + 1],
                )
            # transpose h1gT -> h1g normal [40, 768] (fp32 PE transpose)
            for mt in range(DC):
                tp = ps.tile([BPC * PM, 128], f32, tag="ps", space="PSUM")
                nc.tensor.transpose(tp, h1gT[:, mt, :], ident_f)
                nc.vector.tensor_copy(
                    out=h1g[:, mt * 128 : (mt + 1) * 128], in_=tp
                )
            if apply_gb:
                grow_m = small.tile([1, D], f32, tag="grow")
                brow_m = small.tile([1, D], f32, tag="brow")
                nc.sync.dma_start(out=grow_m, in_=mlng_in[:])
                nc.sync.dma_start(out=brow_m, in_=mlnb_in[:])
                layernorm(h1g[:], grow_m, brow_m, gbtile=gb_sb)
            else:
                layernorm(h1g[:])
            if DBG:
                nc.sync.dma_start(out=dbg_h1gT[:], in_=h1gT)
                nc.sync.dma_start(out=dbg_h2[:], in_=h1g)
                nc.sync.dma_start(out=dbg_hp[:], in_=hp_sb)
            nc.vector.tensor_copy(out=h2bf, in_=h1g)
            for c in range(DC):
                tp = ps.tile([128, BPC * PM], bf16, tag="ps", space="PSUM")
                nc.tensor.transpose(
                    tp, h2bf[:, c * 128 : (c + 1) * 128], ident_b[0 : BPC * PM, 0 : BPC * PM]
                )
                nc.vector.tensor_copy(out=h2T[:, c, :], in_=tp)
            # ---- tied decoder, streaming tok_embed.T ----
            NVT = (V + 511) // 512  # 63 tiles (62x512 + 256)
            for vt in range(NVT):
                n0 = vt * 512
                nn = min(512, V - n0)
                et = etp.tile([128, DC, 512], bf16, tag="et")
                nc.sync.dma_start(
                    out=et[:, :, :nn],
                    in_=embT_in[:, :, n0 : n0 + nn].rearrange("c p v -> p c v"),
                )
                db = lgp.tile([1, 512], bf16, tag="db")
                nc.sync.dma_start(out=db[:, :nn], in_=decb_in[:, n0 : n0 + nn])
                dp = ps.tile([BPC * PM, 512], f32, tag="ps", space="PSUM")
                nc.tensor.matmul(
                    out=dp[:, :nn],
                    lhsT=ones_b[:, 0 : BPC * PM],
                    rhs=db[:, :nn],
                    start=True,
                    stop=False,
                )
                for c in range(DC):
                    nc.tensor.matmul(
                        out=dp[:, :nn],
                        lhsT=h2T[:, c, :],
                        rhs=et[:, c, :nn],
                        start=False,
                        stop=(c == DC - 1),
                    )
                lg = lgp.tile([BPC * PM, 512], f32, tag="lg")
                if vt % 2 == 0:
                    nc.vector.tensor_copy(out=lg[:, :nn], in_=dp[:, :nn])
                else:
                    nc.scalar.copy(out=lg[:, :nn], in_=dp[:, :nn])
                nc.sync.dma_start(out=lg_out[:, n0 : n0 + nn], in_=lg[:, :nn])

    nc.compile()
    return nc


def _get_nc(apply_gb: bool):
    if apply_gb not in _BUILD_CACHE:
        _BUILD_CACHE[apply_gb] = _build(apply_gb)
    return _BUILD_CACHE[apply_gb]


LAST_RESULTS = None


def kernel(**inputs) -> tuple:
    from concourse.bass_utils import run_bass_kernel_spmd

    bf = ml_dtypes.bfloat16
    f32 = np.float32
    ids = np.asarray(inputs["input_ids"]).astype(np.int32)
    seg = np.asarray(inputs["segment_ids"]).astype(np.int32)
    mpos = np.asarray(inputs["masked_pos"]).astype(np.int32)

    def pm(a, dt):  # [b, S] -> [128, b, TT] partition-major
        return np.ascontiguousarray(
            a.reshape(a.shape[0], TT, 128).transpose(2, 0, 1)
        ).astype(dt)

    g = inputs
    trivial = (
        np.all(np.asarray(g["emb_g"]) == 1) and np.all(np.asarray(g["emb_b"]) == 0)
        and np.all(np.asarray(g["ln_g"]) == 1) and np.all(np.asarray(g["ln_b"]) == 0)
        and np.all(np.asarray(g["mlm_ln_g"]) == 1)
        and np.all(np.asarray(g["mlm_ln_b"]) == 0)
    )
    apply_gb = not trivial
    nc = _get_nc(apply_gb)

    wq = np.asarray(g["Wq"], f32).reshape(L, DC, 128, D).astype(bf)
    wk = np.asarray(g["Wk"], f32).reshape(L, DC, 128, D).astype(bf)
    wv = np.asarray(g["Wv"], f32).reshape(L, DC, 128, D).astype(bf)
    wo = np.asarray(g["Wo"], f32).reshape(L, DC, 128, D).astype(bf)
    bq = np.asarray(g["bq"], f32)  # [L, 768]
    bk = np.asarray(g["bk"], f32)
    bqk = np.stack([bq, bk], 1).reshape(L, 2, DC, 128).transpose(3, 0, 1, 2)
    bqk = np.ascontiguousarray(bqk).astype(f32)
    bvo = np.stack([np.asarray(g["bv"], f32), np.asarray(g["bo"], f32)], 1).astype(bf)
    tok = np.asarray(g["tok_embed"], f32)
    embT = np.ascontiguousarray(tok.T).reshape(DC, 128, V).astype(bf)
    fcw = np.asarray(g["fc_W"], f32).reshape(DC, 128, D).astype(bf)
    fcb = np.asarray(g["fc_b"], f32).reshape(DC, 128).T.copy()
    clsw = np.asarray(g["cls_W"], f32).reshape(DC, 128, 2).astype(bf)
    clsb = np.asarray(g["cls_b"], f32).reshape(2, 1)
    mlmw = np.asarray(g["mlm_W"], f32).reshape(DC, 128, D).astype(bf)
    mlmb = np.asarray(g["mlm_b"], f32).reshape(DC, 128).T.copy()
    decb = np.asarray(g["dec_bias"], f32).reshape(1, V).astype(bf)

    shared = dict(
        pos_embed=np.asarray(g["pos_embed"], f32).astype(bf),
        seg_row=np.asarray(g["seg_embed"], f32).astype(bf),
        wq=wq, wk=wk, wv=wv, wo=wo, bqk=bqk, bvo=bvo,
        lng=np.asarray(g["ln_g"], f32), lnb=np.asarray(g["ln_b"], f32),
        embg=np.asarray(g["emb_g"], f32).reshape(1, D),
        embb=np.asarray(g["emb_b"], f32).reshape(1, D),
        fcw=fcw, fcb=fcb, clsw=clsw, clsb=clsb,
        mlmw=mlmw, mlmb=mlmb,
        mlng=np.asarray(g["mlm_ln_g"], f32).reshape(1, D),
        mlnb=np.asarray(g["mlm_ln_b"], f32).reshape(1, D),
        decb=decb, embT=embT,
    )

    in_maps = []
    for c in range(NCORES):
        b0 = c * BPC
        idsc = ids[b0 : b0 + BPC]
        segc = seg[b0 : b0 + BPC]
        sel = np.zeros((BPC, TT, 128, JCOL), f32)
        for b in range(BPC):
            for j in range(PM):
                p = int(mpos[b0 + b, j])
                sel[b, p // 128, p % 128, j] = 1.0
            sel[b, 0, 0, PM] = 1.0
        # one-hot segment selector [NSEG, BPC, TT, 128]
        segsel = np.zeros((NSEG, BPC, TT, 128), f32)
        sc_ = segc.reshape(BPC, TT, 128)
        for s_ in range(NSEG):
            segsel[s_] = (sc_ == s_)
        m = dict(shared)
        m["embtok"] = np.ascontiguousarray(tok[idsc])  # [BPC, S, D] f32 gather
        m["segsel"] = segsel.astype(bf)
        m["maskb"] = pm(np.where(idsc == 0, -30000.0, 0.0), f32)
        m["sel"] = sel.astype(bf)
        in_maps.append(m)

    res = run_bass_kernel_spmd(
        nc,
        in_maps,
        core_ids=list(range(NCORES)),
        trace=bool(int(os.environ.get("BERT_TRACE", "0"))),
    )
    global LAST_RESULTS
    LAST_RESULTS = res

    x = np.concatenate([r["x_out"] for r in res.results], 0).astype(f32)
    lg = np.concatenate(
        [r["lg_out"].reshape(BPC, PM, V) for r in res.results], 0
    ).astype(f32)
    nsp = np.concatenate([r["nsp_out"].T for r in res.results], 0).astype(f32)
    return x, lg, nsp
